# revision 4
# baseline (speedup 1.0000x reference)
"""Trainium2 Bass kernel for nn_KlindtReadoutPerChannel2D — hybrid fp16 + fp8-DR.

Reference computation:
    out[b, n] = sum_{c,p} x[b,c,p] * mask_weights[p,c,n] * readout_weights[c,n]
with B=256, C=64, H=W=36 (P=1296), N=2000.

Why this design: the fp16 baseline (118 us) sits at ~98% of the bf16/fp16 PE
roofline (324k streaming cycles @ ~2.8 GHz), so the only lever left on TRN2 is
the fp8e4/e5 DoubleRow mode (2 MACs/cell/cycle).  Pure e4m3 fails the 2e-2
error gate (measured 3.7e-2), so we split the contraction:

  * k sharded over 8 cores (8 channels/core, K' = 10368 = 81 k-tiles of 128);
    host sums the partial outputs.
  * readout_weights folded into the weights ON HOST with a per-output-column
    scale; x scaled per-core.  Shared scales let fp16 and fp8 partial products
    accumulate in the same PSUM banks, dequantized once on host.
  * First K1 = 81-2*D k-tiles run in fp16 (error-free), last 2*D k-tiles run
    as D DoubleRow supertiles in fp8 e4m3 at 2x rate.  D is chosen so the
    total rel-err stays ~1.7e-2 < 2e-2.
  * Partial outputs leave as fp16 (error ~6e-5 relative, half the DMA).
"""

import numpy as np

B = 256
C = 64
P = 1296  # 36*36
N = 2000
NCORES = 8
CPC = C // NCORES  # channels per core = 8
KTOT = P * CPC  # per-core contraction length = 10368
KT = KTOT // 128  # 81 k-tiles
D = 9  # DoubleRow supertiles per core (2*D k-tiles in e4m3)
K1T = KT - 2 * D  # fp16 k-tiles
NB = 500  # matmul free-dim (PSUM bank holds 512 fp32)
NJ = N // NB  # 4 n-blocks
MT = B // 128  # 2 m-tiles
SW = 14.0  # weight per-column scale target
SX = 14.0  # x per-core scale target

_PROGRAM = {}


def _build_program(repeats=1):
    from contextlib import ExitStack

    from concourse import bacc, mybir, tile

    nc = bacc.Bacc("TRN2", target_bir_lowering=False, debug=False)
    f32 = mybir.dt.float32
    f16 = mybir.dt.float16
    u8 = mybir.dt.uint8
    f8 = mybir.dt.float8e4
    DR = mybir.MatmulPerfMode.DoubleRow

    xt16_d = nc.dram_tensor("xt16", (K1T, 128, B), f16, kind="ExternalInput").ap()
    w16_d = nc.dram_tensor("w16", (K1T, 128, N), f16, kind="ExternalInput").ap()
    xt8_d = nc.dram_tensor("xt8", (D, 128, 2, B), u8, kind="ExternalInput").ap()
    w8_d = nc.dram_tensor("w8", (D, 128, 2, N), u8, kind="ExternalInput").ap()
    out_d = nc.dram_tensor("out", (B, N), f16, kind="ExternalOutput").ap()

    with tile.TileContext(nc) as tc:
        with ExitStack() as ctx:
            w_pool = ctx.enter_context(tc.tile_pool(name="w", bufs=6))
            xt_pool = ctx.enter_context(tc.tile_pool(name="xt", bufs=6))
            out_pool = ctx.enter_context(tc.tile_pool(name="out", bufs=2))
            psum_pool = ctx.enter_context(
                tc.tile_pool(name="psum", bufs=1, space="PSUM")
            )

            # One PSUM tile spanning all 8 banks: bank (m*NJ + j) holds
            # out[m*128:(m+1)*128, j*500:(j+1)*500] (512-aligned slots).
            acc = psum_pool.tile([128, 8 * 512], f32)

            for _rep in range(repeats):
                # fp16 section
                for k in range(K1T):
                    xt_t = xt_pool.tile([128, B], f16)
                    nc.sync.dma_start(xt_t[:], xt16_d[k])
                    w_t = w_pool.tile([128, N], f16)
                    nc.sync.dma_start(w_t[:], w16_d[k])
                    for m in range(MT):
                        lhsT = xt_t[:, m * 128 : (m + 1) * 128]
                        for j in range(NJ):
                            nc.tensor.matmul(
                                acc[:, (m * NJ + j) * 512 : (m * NJ + j) * 512 + NB],
                                lhsT,
                                w_t[:, j * NB : (j + 1) * NB],
                                start=(k == 0),
                                stop=False,
                            )

                # fp8 e4m3 DoubleRow section: D supertiles of 256 rows
                for t in range(D):
                    xt8_t = xt_pool.tile([128, 2, B], u8)
                    nc.sync.dma_start(xt8_t[:], xt8_d[t])
                    w8_t = w_pool.tile([128, 2, N], u8)
                    nc.sync.dma_start(w8_t[:], w8_d[t])
                    last = t == D - 1
                    for m in range(MT):
                        lhsT = xt8_t[:, :, m * 128 : (m + 1) * 128].bitcast(f8)
                        for j in range(NJ):
                            nc.tensor.matmul(
                                acc[:, (m * NJ + j) * 512 : (m * NJ + j) * 512 + NB],
                                lhsT,
                                w8_t[:, :, j * NB : (j + 1) * NB].bitcast(f8),
                                start=False,
                                stop=last,
                                perf_mode=DR,
                            )

                for m in range(MT):
                    for j in range(NJ):
                        o_t = out_pool.tile([128, NB], f16)
                        nc.vector.tensor_copy(
                            o_t[:], acc[:, (m * NJ + j) * 512 : (m * NJ + j) * 512 + NB]
                        )
                        nc.sync.dma_start(
                            out_d[m * 128 : (m + 1) * 128, j * NB : (j + 1) * NB],
                            o_t[:],
                        )

    nc.compile()
    return nc


def _make_in_maps(x, mask_weights, readout_weights):
    import ml_dtypes

    e4 = ml_dtypes.float8_e4m3
    K1 = K1T * 128

    x_flat = np.asarray(x, dtype=np.float32).reshape(B, C, P)
    mask_weights = np.asarray(mask_weights, dtype=np.float32)
    readout_weights = np.asarray(readout_weights, dtype=np.float32)

    in_maps = []
    dequants = []  # per-core (N,) fp64 dequant factors
    for core in range(NCORES):
        cs = slice(core * CPC, (core + 1) * CPC)

        # xt[k, b] with k = p*CPC + c_local (p-major)
        xt = np.ascontiguousarray(x_flat[:, cs, :].transpose(2, 1, 0).reshape(KTOT, B))
        sx = SX / max(np.abs(xt).max(), 1e-30)
        xS = xt * np.float32(sx)
        xt16 = xS[:K1].astype(np.float16).reshape(K1T, 128, B)
        xt8 = np.ascontiguousarray(
            xS[K1:].astype(e4).view(np.uint8).reshape(D, 2, 128, B).transpose(0, 2, 1, 3)
        )

        # w[k, n] = mask * readout, per-column scaled
        w = (mask_weights[:, cs, :] * readout_weights[None, cs, :]).reshape(KTOT, N)
        colmax = np.abs(w).max(axis=0)
        sw = (SW / np.maximum(colmax, 1e-30)).astype(np.float32)
        wS = w * sw[None, :]
        w16 = wS[:K1].astype(np.float16).reshape(K1T, 128, N)
        w8 = np.ascontiguousarray(
            wS[K1:].astype(e4).view(np.uint8).reshape(D, 2, 128, N).transpose(0, 2, 1, 3)
        )

        in_maps.append({"xt16": xt16, "w16": w16, "xt8": xt8, "w8": w8})
        dequants.append(1.0 / (np.float64(sx) * sw.astype(np.float64)))
    return in_maps, dequants


def _get_program(repeats=1):
    if repeats not in _PROGRAM:
        _PROGRAM[repeats] = _build_program(repeats)
    return _PROGRAM[repeats]


def run_sharded(in_maps, **kwargs):
    from concourse.bass_utils import run_bass_kernel_spmd

    nc = _get_program()
    return run_bass_kernel_spmd(nc, in_maps, core_ids=list(range(NCORES)), **kwargs)


def combine_outputs(partials, dequants):
    out = np.zeros((B, N), dtype=np.float64)
    for part, dq in zip(partials, dequants):
        out += np.asarray(part, dtype=np.float64) * dq[None, :]
    return out.astype(np.float32)


def kernel(x, mask_weights, readout_weights):
    in_maps, dequants = _make_in_maps(x, mask_weights, readout_weights)
    res = run_sharded(in_maps)
    return combine_outputs([r["out"] for r in res.results], dequants)
